# revision 15
# baseline (speedup 1.0000x reference)
"""v7c: one fp16 matmul per 4 samples. Host packs samples at 32-col strides so
each 128-col weight load (FWL-eligible) covers 4 samples on aligned strips;
moving streams the 108 real columns; diagonal blocks extracted on DVE+Act."""

import os
import sys

import numpy as np

for _p in (
    "/root/.axon_site",
    "/root/.axon_site/_ro/trn_rl_repo",
    "/opt/trn_rl_repo",
):
    if os.path.isdir(_p) and _p not in sys.path:
        sys.path.append(_p)

import concourse.bacc as bacc
import concourse.mybir as mybir
import concourse.tile as tile
from concourse.ap import AP

NF = 27
NP32 = 32  # per-sample column pitch in the padded layout
D = 128
B = 32768
NCORES = 8
S = B // NCORES

F16 = mybir.dt.float16
F32 = mybir.dt.float32

TOFF = np.concatenate([[0], np.cumsum(NF - np.arange(NF))]).astype(np.int64)
NPAIRS = int(TOFF[NF])
DOUT = D + NPAIRS

PACK = 4  # samples per matmul (4 x 32 = 128 weight cols)
PPT = 16  # packs per PSUM tile (16 x 4 x 27 = 1728 fp32 = 4 banks)
NSPLIT = 1  # column-split count for each chunk's input DMA


def build_nc(s_per_core=S):
    c_sz = 512  # samples per chunk
    nchunks = s_per_core // c_sz
    packs_per_chunk = c_sz // PACK  # 128
    tiles_per_chunk = packs_per_chunk // PPT  # 16

    nc = bacc.Bacc("TRN2", target_bir_lowering=False, debug=False)
    xt = nc.dram_tensor("xt", [D, s_per_core * NP32], F16, kind="ExternalInput")
    gram = nc.dram_tensor(
        "gram", [PACK, NF, s_per_core // PACK, NF], F16, kind="ExternalOutput"
    )

    with tile.TileContext(nc) as tc:
        with (
            tc.tile_pool(name="xin", bufs=3) as xin_pool,
            tc.tile_pool(name="gbuf", bufs=2) as gbuf_pool,
            tc.tile_pool(name="ps", bufs=2, space="PSUM") as ps_pool,
        ):
            dma_engines = [nc.gpsimd, nc.sync, nc.scalar]
            rr = [0]
            cc = [0]
            for c0 in range(nchunks):
                gbuf = gbuf_pool.tile([128, packs_per_chunk * NF], F16)
                xin = xin_pool.tile([D, c_sz * NP32], F16)
                csp = c_sz * NP32 // NSPLIT
                for sp in range(NSPLIT):
                    eng = dma_engines[rr[0] % 3]
                    rr[0] += 1
                    eng.dma_start(
                        out=xin[:, sp * csp : (sp + 1) * csp],
                        in_=xt[
                            :,
                            c0 * c_sz * NP32 + sp * csp : c0 * c_sz * NP32
                            + (sp + 1) * csp,
                        ],
                    )
                for t in range(tiles_per_chunk):
                    ps = ps_pool.tile([128, PPT * PACK * NF], F32)
                    for q in range(PPT):
                        loc = (t * PPT + q) * PACK * NP32
                        wsl = xin[:, loc : loc + 128]
                        # moving: skip the 5 pad cols of each 32-col block
                        msl = AP(
                            wsl.tensor,
                            wsl.offset,
                            [list(wsl.ap[0]), [NP32, PACK], [1, NF]],
                        )
                        nc.tensor.matmul(
                            ps[:, q * PACK * NF : (q + 1) * PACK * NF],
                            wsl,
                            msl,
                            start=True,
                            stop=True,
                        )
                    # extract diagonal 27x27 blocks: strip 32l holds sample 4p+l
                    for l in range(PACK):
                        src = ps[32 * l : 32 * l + NF, :].rearrange(
                            "p (q s m) -> p q s m", q=PPT, s=PACK
                        )[:, :, l]
                        dst = gbuf[
                            32 * l : 32 * l + NF,
                            t * PPT * NF : (t + 1) * PPT * NF,
                        ].rearrange("p (q m) -> p q m", q=PPT)
                        if cc[0] % 2 == 0:
                            nc.vector.tensor_copy(dst, src)
                        else:
                            nc.scalar.copy(dst, src)
                        cc[0] += 1
                for l in range(PACK):
                    eng2 = dma_engines[rr[0] % 3]
                    rr[0] += 1
                    eng2.dma_start(
                        out=gram[
                            l,
                            :,
                            c0 * packs_per_chunk : (c0 + 1) * packs_per_chunk,
                            :,
                        ],
                        in_=gbuf[32 * l : 32 * l + NF, : packs_per_chunk * NF],
                    )
    nc.finalize()
    return nc


def host_pack_inputs(dense_features, sparse_features):
    bsz = dense_features.shape[0]
    xt = np.zeros((D, bsz, NP32), dtype=np.float16)
    xt[:, :, 0] = np.asarray(dense_features, dtype=np.float32).T
    xt[:, :, 1:NF] = np.asarray(sparse_features, dtype=np.float32).transpose(2, 0, 1)
    return xt


def host_core_input(xt, c, s_per_core=S):
    return np.ascontiguousarray(
        xt[:, c * s_per_core : (c + 1) * s_per_core, :]
    ).reshape(D, s_per_core * NP32)


def host_unpack_output(dense_features, gram_t):
    """gram_t: [PACK, NF, B//PACK, NF]; sample 4k+l lives at gram_t[l, :, k, :]."""
    bsz = dense_features.shape[0]
    out = np.empty((bsz, DOUT), dtype=np.float32)
    out[:, :D] = dense_features
    gram_t = gram_t.astype(np.float32)
    for l in range(PACK):
        for n in range(NF):
            lo = D + int(TOFF[n])
            out[l::PACK, lo : lo + NF - n] = gram_t[l, n, :, n:]
    return out


_NC_CACHE = {}


def _get_nc():
    key = (S,)
    if key not in _NC_CACHE:
        _NC_CACHE[key] = build_nc(S)
    return _NC_CACHE[key]


def kernel(dense_features, sparse_features):
    from concourse.bass_utils import run_bass_kernel_spmd

    dense_features = np.asarray(dense_features, dtype=np.float32)
    sparse_features = np.asarray(sparse_features, dtype=np.float32)
    xt = host_pack_inputs(dense_features, sparse_features)
    in_maps = [{"xt": host_core_input(xt, c)} for c in range(NCORES)]
    nc = _get_nc()
    res = run_bass_kernel_spmd(nc, in_maps, core_ids=list(range(NCORES)))
    gram_t = np.concatenate([r["gram"] for r in res.results], axis=2)
    return host_unpack_output(dense_features, gram_t)
